# revision 42
# baseline (speedup 1.0000x reference)
"""Trainium2 Bass kernel for the capsule-routing module.

Full-input contract: kernel(**inputs) takes the full [32,...] inputs,
shards batch over 8 NeuronCores (4 per core), runs the Bass kernel via
run_bass_kernel_spmd, and concatenates per-core outputs.

Math (per core, BL=4 local batches):
  The reference computes Wn = einsum('nck,kio->ncio', alpha, W) (1 GB) and
  u_hat = einsum('bni,ncio->bcno', x, Wn).  We never materialize either.
  With G[n,(k,c)] = c_route[b,c,n] * alpha[n,c,k]:
    v[b,c,o]   = sum_{k,i} W[k,i,o] * hT[b][i,(k,c)],
                 hT[b][i,(k,c)] = sum_n x[b,n,i] * G[b][n,(k,c)]
    a[b,c,n]   = sum_k alpha[n,c,k] * e[b][(k,c),n],
                 e[b][(k,c),n] = sum_i wv[b][i,(k,c)] * xT[b][i,n]
                 wv[b][i,(k,c)] = sum_o W[k,i,o] * v[b,c,o]
  All matmuls run in float32r (FP22 inputs, fp32 accumulation): routing
  logit increments get ~1e-3 absolute error on an O(100) scale, far inside
  the softmax's tolerance.  The squash factor sqrt(sn)/(1+sn) is computed
  as exp(0.5*ln(sn) - ln(1+sn)) so the scalar engine only ever needs the
  ln/exp table (no ACT_TABLE_LOAD thrash), and in routing passes it is
  applied AFTER the wv/e matmuls (folded into the e*alpha elementwise
  multiply via a per-(k,c)-row factor), so wv/e never wait on the squash
  chain.  Routing logits live in a persistent PSUM tile that the per-node
  logit matmuls accumulate into across passes (start only at pass 0).
"""

import sys

sys.path.insert(0, "/opt/trn_rl_repo")

from contextlib import ExitStack

import numpy as np

import concourse.bacc as bacc
import concourse.mybir as mybir
import concourse.tile as tile

F32 = mybir.dt.float32
FR = mybir.dt.float32r
AX = mybir.AxisListType
ALU = mybir.AluOpType
ACTF = mybir.ActivationFunctionType

B, NODES, IN_DIM, OUT_DIM, CAPS, K, NUM_ROUTE = 32, 512, 256, 128, 16, 5, 3
NCORES = 8
BL = B // NCORES          # 4 batches per core
NCH = NODES // 128        # 4 node chunks
IH = IN_DIM // 128        # 2 input-dim chunks
Q = K * CAPS              # 80 = (k,c) packed, q = k*16 + c
NC10 = K * IH             # 10 contraction chunks over (k, ih)
NG = BL * NCH             # 16 softmax groups (b, nch)
BC = BL * CAPS            # 64


def caps_kernel(ctx, tc, out_d, x_d, xt_d, w2_d, w2t_d, a2g_d, g0_d,
                ae_d, ssel_d, ident_d, ones_d, rcsel_d, fmask_d):
    nc = tc.nc

    sb = ctx.enter_context(tc.tile_pool(name="sb", bufs=1))
    work = ctx.enter_context(tc.tile_pool(name="work", bufs=2))
    ps_log = ctx.enter_context(tc.tile_pool(name="ps_log", bufs=1, space="PSUM"))
    ps_h = ctx.enter_context(tc.tile_pool(name="ps_h", bufs=2, space="PSUM"))
    ps_e = ctx.enter_context(tc.tile_pool(name="ps_e", bufs=2, space="PSUM"))
    ps_wv = ctx.enter_context(tc.tile_pool(name="ps_wv", bufs=1, space="PSUM"))
    ps_s = ctx.enter_context(tc.tile_pool(name="ps_s", bufs=1, space="PSUM"))

    # ---------------- persistent SBUF ----------------
    ident = sb.tile([128, 128], FR, tag="ident")
    ones_col = sb.tile([128, 1], FR, tag="ones_col")
    ones4 = sb.tile([128, 4], FR, tag="ones4")   # fp32r mm needs free >= 2
    ones_row = sb.tile([1, 128], FR, tag="ones_row")

    x_sb = sb.tile([128, NG * IN_DIM], FR, tag="x_sb")          # [p, (b,j,i)]
    xt_sb = sb.tile([128, BL * IH * NODES], FR, tag="xt_sb")    # [i, (b,ih,n)]
    w2 = sb.tile([128, NC10 * 128], FR, tag="w2")               # [i, (c10,o)]
    w2t = sb.tile([128, NC10 * 128], FR, tag="w2t")             # [o, (c10,ki)]
    a2g = sb.tile([128, NCH * Q], F32, tag="a2g")               # [p, (j,k,c)]
    a_e = sb.tile([Q, NODES], F32, tag="a_e")                   # [q, n]
    s_sel = sb.tile([Q, CAPS], FR, tag="s_sel")                 # [q, c]
    rc_sel = sb.tile([BC, Q], FR, tag="rc_sel")                 # [(b,c), q]
    fmask = sb.tile([BC, BL], F32, tag="fmask")                 # [(b,c), b]
    g0 = sb.tile([128, NCH * Q], FR, tag="g0")                  # iter-0 G
    warm = sb.tile([1, 1], F32, tag="warm")
    # routing logits live in SBUF: [p, (b,j,c)]; per-pass increments are
    # matmul'd into a scratch PSUM bank then added on DVE (PSUM does not
    # accumulate across closed matmul groups)
    logits = sb.tile([128, NG * CAPS], F32, tag="logits")
    # one shared PSUM bank for all small matmul/transpose outputs
    # (per-tag psum slots are bank-granular, so slicing one tile by hand
    # is the only way to fit everything in the 8 banks)
    small = ps_s.tile([128, 512], F32, tag="small")
    sm_htp = [small[:, 0:Q].bitcast(FR), small[:, 80:80 + Q].bitcast(FR)]
    sm_vps = small[:, 160:160 + BC]
    sm_snq4 = small[:BC, 224:228]
    sm_snq = small[:BC, 224:225]
    sm_facq = small[:Q, 228:228 + BL]
    sm_snp = small[0:1, 232:232 + BC]
    sm_fbp = small[:, 296:296 + BC]
    sm_outp = small[:BC, 360:360 + 128].bitcast(FR)

    # ---------------- input DMA ----------------
    # Pass-0 critical tensors first (a2g -> g0, x, w2); the rest streams in
    # under pass-0 compute.  The contribution input is dropped: softmax over
    # caps is invariant to the per-(b,n) constant it adds.
    def load_x(b):
        for j in range(NCH):
            nc.sync.dma_start(
                x_sb[:, (b * NCH + j) * IN_DIM:(b * NCH + j + 1) * IN_DIM],
                x_d[b, j * 128:(j + 1) * 128, :],
            )

    # pass-0 critical path first: x(b0) + host-prepped g0 (split over
    # queues) + ident; the rest ordered by first use (a2g/s_sel/a_e only
    # needed from the pass-0 tail onward)
    load_x(0)
    for j in range(NCH):
        nc.sync.dma_start(g0[:, j * Q:(j + 1) * Q],
                          g0_d[:, j * Q:(j + 1) * Q])
    nc.sync.dma_start(ident[:], ident_d[:, :])   # h transposes
    load_x(1)
    load_x(2)
    load_x(3)
    nc.sync.dma_start(w2[:], w2_d[:, :])         # v
    nc.sync.dma_start(ones_col[:], ones_d[:, 0:1])
    nc.sync.dma_start(ones4[:], ones_d[:, 0:4])
    nc.sync.dma_start(ones_row[:1, :], ones_d[0:1, :].rearrange("a p -> a p"))
    nc.sync.dma_start(rc_sel[:BC, :], rcsel_d[:, :])
    nc.sync.dma_start(fmask[:BC, :], fmask_d[:, :])
    nc.sync.dma_start(w2t[:], w2t_d[:, :])       # wv
    for b in range(BL):
        nc.sync.dma_start(
            xt_sb[:, b * IH * NODES:(b + 1) * IH * NODES],
            xt_d[:, b * IH * NODES:(b + 1) * IH * NODES],
        )
    nc.sync.dma_start(a_e[:Q, :], ae_d[:, :])
    for j in range(NCH):
        nc.sync.dma_start(a2g[:, j * Q:(j + 1) * Q],
                          a2g_d[:, j * Q:(j + 1) * Q])
    nc.sync.dma_start(s_sel[:Q, :], ssel_d[:, :])

    # warm the ln/exp activation table while DMA streams in
    nc.any.memset(warm[:1, :1], 1.0)
    nc.scalar.activation(warm[:1, :1], warm[:1, :1], ACTF.Ln)
    nc.gpsimd.memset(logits[:], 0.0)

    # ---------------- routing ----------------
    # GPSIMD (Pool) cannot touch PSUM: alternate DVE / Act for psum drains
    def spread_copy(idx, dst, src):
        if idx % 2 == 1:
            nc.scalar.copy(dst, src)
        else:
            nc.vector.tensor_copy(dst, src)

    def alloc_softmax():
        return {
            "mx": work.tile([128, NG], F32, tag="mx", name="mx"),
            "sub": work.tile([128, NG * CAPS], F32, tag="sub", name="sub"),
            "exp": work.tile([128, NG * CAPS], F32, tag="exp", name="exp"),
            "sm": work.tile([128, NG], F32, tag="sm", name="sm"),
            "rc": work.tile([128, NG], F32, tag="rc", name="rc"),
            "gt": work.tile([128, NG * Q], FR, tag="gt", name="gt"),
        }

    def emit_softmax_b(b, s):
        # softmax over caps for one batch + fused G build (G = exp*rc*alpha)
        mx, sub, exp, sm, rc, gt = (s["mx"], s["sub"], s["exp"], s["sm"],
                                    s["rc"], s["gt"])
        gs = slice(b * NCH, (b + 1) * NCH)
        cs = slice(b * NCH * CAPS, (b + 1) * NCH * CAPS)
        lg3 = logits[:, cs].rearrange("p (g c) -> p g c", g=NCH)
        nc.vector.reduce_max(mx[:, gs], lg3, axis=AX.X)
        nc.vector.tensor_sub(
            sub[:, cs].rearrange("p (g c) -> p g c", g=NCH),
            lg3,
            mx[:, gs].unsqueeze(2).broadcast_to([128, NCH, CAPS]),
        )
        nc.scalar.activation(exp[:, cs], sub[:, cs], ACTF.Exp)
        nc.vector.reduce_sum(
            sm[:, gs],
            exp[:, cs].rearrange("p (g c) -> p g c", g=NCH),
            axis=AX.X)
        nc.vector.reciprocal(rc[:, gs], sm[:, gs])
        for j in range(NCH):
            g = b * NCH + j
            # TensorScalarPtr is DVE-only (invalid opcode on Pool)
            nc.vector.scalar_tensor_tensor(
                gt[:, g * Q:(g + 1) * Q].rearrange("p (k c) -> p k c", k=K),
                exp[:, g * CAPS:(g + 1) * CAPS]
                .unsqueeze(1).broadcast_to([128, K, CAPS]),
                rc[:, g:g + 1],
                a2g[:, j * Q:(j + 1) * Q].rearrange("p (k c) -> p k c", k=K),
                op0=ALU.mult, op1=ALU.mult,
            )

    cur = None   # softmax tiles for the current pass (None => uniform g0)
    for t in range(NUM_ROUTE + 1):
        fin = (t == NUM_ROUTE)
        if cur is None:
            def g_slice(b, j):
                return g0[:, j * Q:(j + 1) * Q]
        else:
            def g_slice(b, j, gt=cur["gt"]):
                return gt[:, (b * NCH + j) * Q:(b * NCH + j + 1) * Q]

        # --- h[b] = G_b^T @ x_b : psum [q(80) x i(256)] per b, then
        # --- PE-transpose the two i-halves into ht_sb [i(128), (b, ih, q)] ---
        ht_sb = work.tile([128, BL * IH * Q], FR, tag="ht")
        for b in range(BL):
            hps = ps_h.tile([Q, IN_DIM], F32, tag="hps")
            for j in range(NCH):
                nc.tensor.matmul(
                    hps[:Q, :],
                    g_slice(b, j),
                    x_sb[:, (b * NCH + j) * IN_DIM:
                         (b * NCH + j + 1) * IN_DIM],
                    start=(j == 0),
                    stop=(j == NCH - 1),
                )
            h_sb = work.tile([Q, IN_DIM], FR, tag="h")
            nc.scalar.copy(h_sb[:Q, :], hps[:Q, :])
            for ih in range(IH):
                htp = sm_htp[ih]
                nc.tensor.transpose(
                    htp,
                    h_sb[:Q, ih * 128:(ih + 1) * 128],
                    ident[:Q, :Q],
                )
                spread_copy(b * IH + ih,
                            ht_sb[:, (b * IH + ih) * Q:(b * IH + ih + 1) * Q],
                            htp)

        # --- V[o, (b,c)] = sum_{k,i} W2[(ki),o] * hT[b][i,(k,c)] ---
        vps = sm_vps
        ht_v = ht_sb[:].rearrange("p (b ih q) -> p b ih q", b=BL, ih=IH)
        for c10 in range(NC10):
            k, ih = divmod(c10, IH)
            nc.tensor.matmul(
                vps.rearrange("p (b c) -> p b c", b=BL),
                w2[:, c10 * 128:(c10 + 1) * 128],
                ht_v[:, :, ih, k * CAPS:(k + 1) * CAPS],
                start=(c10 == 0),
                stop=(c10 == NC10 - 1),
            )
        v_sb = work.tile([128, BC], FR, tag="v_sb")
        nc.vector.tensor_copy(v_sb[:], vps)

        # --- squash factor fac = sqrt(sn)/(1+sn) = exp(.5*ln(sn)-ln(1+sn)),
        # --- computed off the critical path (wv/e use the UNSCALED v) ---
        sq = work.tile([128, BC], FR, tag="sq")
        nc.gpsimd.tensor_mul(sq[:], v_sb[:], v_sb[:])

        if fin:
            # row-layout factor for partition-broadcast onto v
            snp = sm_snp
            nc.tensor.matmul(snp, ones_col[:], sq[:],
                             start=True, stop=True)
            lnsn = work.tile([1, BC], F32, tag="lnsn2")
            nc.scalar.activation(lnsn[:1, :], snp, ACTF.Ln)
            ln1p = work.tile([1, BC], F32, tag="ln1p2")
            nc.scalar.activation(ln1p[:1, :], snp, ACTF.Ln, bias=1.0)
            arg = work.tile([1, BC], F32, tag="arg2")
            nc.vector.scalar_tensor_tensor(arg[:1, :], lnsn[:1, :], 0.5,
                                           ln1p[:1, :],
                                           op0=ALU.mult, op1=ALU.subtract)
            facr = work.tile([1, BC], FR, tag="facr")
            nc.scalar.activation(facr[:1, :], arg[:1, :], ACTF.Exp)
            fbp = sm_fbp
            nc.tensor.matmul(fbp, ones_row[:1, :], facr[:1, :],
                             start=True, stop=True)
            vsq = work.tile([128, BC], FR, tag="vsq")
            nc.vector.tensor_mul(vsq[:], v_sb[:], fbp)
            outp = sm_outp
            nc.tensor.transpose(outp, vsq[:], ident[:])
            out_sb = work.tile([BC, 128], F32, tag="outsb")
            nc.vector.tensor_copy(out_sb[:BC, :], outp)
            nc.sync.dma_start(
                out_d.rearrange("b c o -> (b c) o"),
                out_sb[:BC, :],
            )
            break

        # column-layout factor: snq[(b,c),1] via sq^T @ ones, then expand to
        # facq[q, b] = fac[b, c(q)] with a constant selector matmul
        nc.tensor.matmul(sm_snq4, sq[:], ones4[:], start=True, stop=True)
        lnsn = work.tile([BC, 1], F32, tag="lnsn")
        nc.scalar.activation(lnsn[:BC, :], sm_snq, ACTF.Ln)
        ln1p = work.tile([BC, 1], F32, tag="ln1p")
        nc.scalar.activation(ln1p[:BC, :], sm_snq, ACTF.Ln, bias=1.0)
        arg = work.tile([BC, 1], F32, tag="arg")
        nc.vector.scalar_tensor_tensor(arg[:BC, :], lnsn[:BC, :], 0.5,
                                       ln1p[:BC, :],
                                       op0=ALU.mult, op1=ALU.subtract)
        facx = work.tile([BC, 1], FR, tag="facx")
        nc.scalar.activation(facx[:BC, :], arg[:BC, :], ACTF.Exp)
        rhsm = work.tile([BC, BL], FR, tag="rhsm")
        nc.vector.tensor_mul(rhsm[:BC, :],
                             facx[:BC, 0:1].broadcast_to([BC, BL]),
                             fmask[:BC, :])
        facq = sm_facq
        nc.tensor.matmul(facq, rc_sel[:BC, :], rhsm[:BC, :],
                         start=True, stop=True)

        # --- wv[i, (k,b,c)] = sum_o W[k,i,o] * v[o, (b,c)] (unscaled) ---
        wvp = ps_wv.tile([128, NC10 * BC], F32, tag="wvp")
        for c10 in range(NC10):
            nc.tensor.matmul(
                wvp[:, c10 * BC:(c10 + 1) * BC],
                w2t[:, c10 * 128:(c10 + 1) * 128],
                v_sb[:],
                start=True, stop=True,
            )
        wv_sb = work.tile([128, IH * BL * Q], FR, tag="wv")
        wvp_v = wvp[:].rearrange("p (k ih b c) -> p ih b k c",
                                 k=K, ih=IH, b=BL)
        for b in range(BL):
            for ih in range(IH):
                spread_copy(b * IH + ih,
                            wv_sb[:, (ih * BL + b) * Q:(ih * BL + b + 1) * Q]
                            .rearrange("p (k c) -> p k c", k=K),
                            wvp_v[:, ih, b])

        # --- e[b] = wv_b^T @ xT_b : [q(80) x n(512)];
        # --- tmp = e * fac[b,c(q)] * alpha; logits += tmp^T @ s_sel;
        # --- then immediately emit the NEXT pass's softmax for this b so it
        # --- overlaps the remaining batches' tail work ---
        aps = ps_log.tile([128, NG * CAPS], F32, tag="aps")
        nxt = alloc_softmax()
        for b in range(BL):
            eps = ps_e.tile([Q, NODES], F32, tag="eps")
            for ih in range(IH):
                nc.tensor.matmul(
                    eps[:Q, :],
                    wv_sb[:, (ih * BL + b) * Q:(ih * BL + b + 1) * Q],
                    xt_sb[:, (b * IH + ih) * NODES:
                          (b * IH + ih + 1) * NODES],
                    start=(ih == 0),
                    stop=(ih == IH - 1),
                )
            tmp = work.tile([Q, NODES], FR, tag="tmp")
            nc.vector.scalar_tensor_tensor(tmp[:Q, :], eps[:Q, :],
                                           facq[:Q, b:b + 1], a_e[:Q, :],
                                           op0=ALU.mult, op1=ALU.mult)
            for j in range(NCH):
                g = b * NCH + j
                nc.tensor.matmul(
                    aps[:, g * CAPS:(g + 1) * CAPS],
                    tmp[:Q, j * 128:(j + 1) * 128],
                    s_sel[:Q, :],
                    start=True, stop=True,
                )
            bs = slice(b * NCH * CAPS, (b + 1) * NCH * CAPS)
            nc.vector.tensor_add(logits[:, bs], logits[:, bs], aps[:, bs])
            emit_softmax_b(b, nxt)
        cur = nxt


_CACHE = {}


def _force_combined_exp_ln_table(arch):
    """Make natural_log_exp_and_others the only act set offering Exp/Ln so
    the table-load pass never alternates tables between softmax (Exp) and
    the squash factor (Ln).  Mutates the functools.cache'd dict in place;
    set indices are untouched so emitted act_func_set_ids stay valid."""
    from concourse.hw_specs import get_activation_tables
    try:
        tabs = get_activation_tables(arch)
    except Exception:
        return
    for name, s in tabs.items():
        if name != "natural_log_exp_and_others":
            s.discard(ACTF.Exp)
            s.discard(ACTF.Ln)


def _build():
    if "nc" in _CACHE:
        return _CACHE["nc"]
    nc = bacc.Bacc("TRN2", target_bir_lowering=False, debug=False,
                   num_devices=NCORES)
    _force_combined_exp_ln_table(nc.m.arch)
    x_d = nc.dram_tensor("x", [BL, NODES, IN_DIM], FR, kind="ExternalInput")
    xt_d = nc.dram_tensor("xt", [128, BL * IH * NODES], FR,
                          kind="ExternalInput")
    w2_d = nc.dram_tensor("w2", [128, NC10 * 128], FR, kind="ExternalInput")
    w2t_d = nc.dram_tensor("w2t", [128, NC10 * 128], FR,
                           kind="ExternalInput")
    a2g_d = nc.dram_tensor("a2g", [128, NCH * Q], F32, kind="ExternalInput")
    g0_d = nc.dram_tensor("g0", [128, NCH * Q], FR, kind="ExternalInput")
    ae_d = nc.dram_tensor("a_e", [Q, NODES], F32, kind="ExternalInput")
    ssel_d = nc.dram_tensor("s_sel", [Q, CAPS], FR, kind="ExternalInput")
    ident_d = nc.dram_tensor("ident", [128, 128], FR, kind="ExternalInput")
    ones_d = nc.dram_tensor("ones", [128, 128], FR, kind="ExternalInput")
    rcsel_d = nc.dram_tensor("rc_sel", [BC, Q], FR, kind="ExternalInput")
    fmask_d = nc.dram_tensor("fmask", [BC, BL], F32, kind="ExternalInput")
    out_d = nc.dram_tensor("out", [BL, CAPS, OUT_DIM], F32,
                           kind="ExternalOutput")
    with tile.TileContext(nc) as tc:
        with ExitStack() as ctx:
            caps_kernel(ctx, tc, out_d.ap(), x_d.ap(),
                        xt_d.ap(), w2_d.ap(), w2t_d.ap(), a2g_d.ap(),
                        g0_d.ap(), ae_d.ap(), ssel_d.ap(), ident_d.ap(),
                        ones_d.ap(), rcsel_d.ap(), fmask_d.ap())
    nc.compile()
    _CACHE["nc"] = nc
    return nc


def host_prep(W, alpha):
    """Constant input layouts shared by all cores."""
    W = np.asarray(W, dtype=np.float32)
    alpha = np.asarray(alpha, dtype=np.float32)
    w2 = np.ascontiguousarray(
        W.reshape(K, IH, 128, OUT_DIM).transpose(2, 0, 1, 3)
        .reshape(128, NC10 * 128))
    w2t = np.ascontiguousarray(
        W.reshape(K, IH, 128, OUT_DIM).transpose(3, 0, 1, 2)
        .reshape(128, NC10 * 128))
    a2g = np.ascontiguousarray(
        alpha.reshape(NCH, 128, CAPS, K).transpose(1, 0, 3, 2)
        .reshape(128, NCH * Q))
    a_e = np.ascontiguousarray(
        alpha.transpose(2, 1, 0).reshape(Q, NODES))
    s_sel = np.ascontiguousarray(
        np.tile(np.eye(CAPS, dtype=np.float32), (K, 1)))
    ident = np.eye(128, dtype=np.float32)
    ones = np.ones((128, 128), dtype=np.float32)
    # rc_sel[(b',c'), q] = [c' == q % CAPS]; fmask[(b',c'), b] = [b' == b]
    cidx = np.arange(BC) % CAPS
    rc_sel = (cidx[:, None] == (np.arange(Q) % CAPS)[None, :]) \
        .astype(np.float32)
    bidx = np.arange(BC) // CAPS
    fmask = (bidx[:, None] == np.arange(BL)[None, :]).astype(np.float32)
    g0 = np.ascontiguousarray(a2g * (1.0 / CAPS))
    return {"w2": w2, "w2t": w2t, "a2g": a2g, "g0": g0, "a_e": a_e,
            "s_sel": s_sel, "ident": ident, "ones": ones, "rc_sel": rc_sel,
            "fmask": fmask}


def prep_xt(xl):
    """Per-core xT layout [i_local(128), (b, ih, n)]."""
    return np.ascontiguousarray(
        xl.reshape(BL, NODES, IH, 128).transpose(3, 0, 2, 1)
        .reshape(128, BL * IH * NODES))


def make_in_maps(x, W, alpha):
    consts = host_prep(W, alpha)
    in_maps = []
    for c in range(NCORES):
        xl = np.ascontiguousarray(np.asarray(x, dtype=np.float32)
                                  [c * BL:(c + 1) * BL])
        in_maps.append({"x": xl, "xt": prep_xt(xl), **consts})
    return in_maps


def _enable_ldw_opt():
    from concourse import bass_utils as bu
    if getattr(bu, "_ldw_patched", False):
        return
    orig = bu.run_command

    def run_command_ldw(argv, **kw):
        argv = ["--enable-ldw-opt=true" if a == "--enable-ldw-opt=false"
                else a for a in argv]
        return orig(argv, **kw)

    bu.run_command = run_command_ldw
    bu._ldw_patched = True


def kernel(x, contribution, W, alpha):
    from concourse import bass_utils
    _enable_ldw_opt()

    nc = _build()
    in_maps = make_in_maps(x, W, alpha)
    res = bass_utils.run_bass_kernel_spmd(nc, in_maps,
                                          core_ids=list(range(NCORES)))
    return np.concatenate([res.results[c]["out"] for c in range(NCORES)],
                          axis=0)
